# revision 28
# baseline (speedup 1.0000x reference)
"""Single-head cross-attention (layernorm + QKV proj + softmax(QK^T)V) on 8 NeuronCores.

Sharding: data-parallel over batch B=8, one batch element per core.

Round 6 — transpose-free device program + fp8 DoubleRow q/k projections.

The round-3/4 kernels DMA-transposed the normalized inputs on-device; the
2-byte-scatter transpose path runs at ~21 GB/s aggregate (~12us per 256KB
tile), pacing chunk delivery at ~44us/group vs the PE's 41us/group — the
projections could never be fed on time. Round 5 removed every on-device
transpose; round 6 additionally runs the q/k projections in fp8e4 DoubleRow
(host ships x*16 and w_centered*2048 as fp8, d-block pairs plane-interleaved;
ACT dequant scale 2^-10), cutting the q/k proj PE time ~1.6x for ~+0.66e-2
rel err (measured 1.86e-2 vs the 2e-2 gate). Structure:

  - The host ships xT (= x^T, [ic, p, db, tok] chunk-major) alongside x.
    Linear DMA, ~3us per 1MB chunk.
  - LN mean-centering is folded into the weights EXACTLY:
      (x - mu) @ W^T  ==  x @ (C W^T),  C = I - J/D  (host: subtract the
    per-output column mean of W^T over d). No apply stage on device.
  - The remaining LN factor rstd multiplies each token row. It is applied:
      v: at dequant, psum[tok, e] * rstd_tok (per-partition stt, as before)
      k: NOT AT ALL at dequant — scores^T psum has k-tokens on partitions,
         so exp(scale_j * s) with scale AP = EXP_SCALE * rstd_k applies it
         exactly. b_k is dropped: it shifts s_ij by f(i) only, which softmax
         over j cancels.
      q: psum[e, tok] needs a per-free-column scale: DVE tensor_tensor with
         rstd_bc (rstd_t broadcast across partitions, built on the PE via
         identity-matmul row-extraction + ones x row outer product).
  - x (token-major) is still loaded for bn_stats -> rstd only; its tiles
    free right after stats, so the stream never backs up.
  - out stored bf16 (host upcasts + divides by Z), fp8e4 DoubleRow scores,
    Z = ones^T @ attn^T on PE, softmax division on host — as round 3/4.
"""

import os
from contextlib import ExitStack

import numpy as np
import ml_dtypes

import concourse.bass as bass
import concourse.bacc as bacc
import concourse.mybir as mybir
import concourse.tile as tile
from concourse.bass import ts, ds
from concourse.bass_utils import run_bass_kernel_spmd

BF16 = mybir.dt.bfloat16
F32 = mybir.dt.float32
FP8 = mybir.dt.float8e4

B, T, D = 8, 2048, 1024
EPS = 1e-5
QK_SCALE = 32.0                 # fp8 quantization scale for q and k
EXP_SCALE = float(D) ** -0.5 / (QK_SCALE * QK_SCALE)   # = 2^-15
P = 128
N_IT = T // P          # 16 token tiles of 128
N_DB = D // P          # 8 d-blocks of 128
N_EB = D // P          # 8 e-blocks of 128
N_IC = T // 512        # 4 token chunks of 512
N_EC = D // 512        # 2 e chunks of 512
USE_FP8_SCORES = bool(int(os.environ.get("KERNEL_FP8", "1")))


def build_module() -> bass.Bass:
    nc = bacc.Bacc("TRN2", target_bir_lowering=False)

    # token-major copies (bn_stats only)
    x_t = nc.dram_tensor("x_t", [T, D], BF16, kind="ExternalInput")
    x_k = nc.dram_tensor("x_k", [T, D], BF16, kind="ExternalInput")
    x_v = nc.dram_tensor("x_v", [T, D], BF16, kind="ExternalInput")
    # host-pretransposed, chunk-major. t/k are fp8 (x16) with d-block PAIRS
    # plane-interleaved for DoubleRow: xT[ic, p, dbp, h, i] = x[ic*512+i,
    # (2*dbp+h)*128+p]. v stays bf16: xT_v[ic, p, db, i].
    xT_t = nc.dram_tensor("xT_t", [N_IC, P, N_DB // 2, 2, 512], FP8,
                          kind="ExternalInput")
    xT_k = nc.dram_tensor("xT_k", [N_IC, P, N_DB // 2, 2, 512], FP8,
                          kind="ExternalInput")
    xT_v = nc.dram_tensor("xT_v", [N_IC, P, N_DB, 512], BF16, kind="ExternalInput")
    # centered+folded weights. q/k fp8 (x2048), pair-interleaved to match:
    # wq/wk[p, eb, dbp, h, el]; v bf16 [p, ec, db, el].
    wq = nc.dram_tensor("wq", [P, N_EB, N_DB // 2, 2, P], FP8,
                        kind="ExternalInput")
    wk = nc.dram_tensor("wk", [P, N_EB, N_DB // 2, 2, P], FP8,
                        kind="ExternalInput")
    wv = nc.dram_tensor("wv", [P, N_EC, N_DB, 512], BF16, kind="ExternalInput")
    bq = nc.dram_tensor("bq", [P, N_EB], F32, kind="ExternalInput")   # x32
    bv = nc.dram_tensor("bv", [P, D], F32, kind="ExternalInput")      # broadcast
    ident = nc.dram_tensor("ident", [P, P], F32, kind="ExternalInput")
    out = nc.dram_tensor("out", [T, D], BF16, kind="ExternalOutput")
    out_z = nc.dram_tensor("out_z", [N_IC, 512], F32, kind="ExternalOutput")

    qk_dt = FP8 if USE_FP8_SCORES else BF16

    with tile.TileContext(nc) as tc, ExitStack() as ctx:
        const = ctx.enter_context(tc.tile_pool(name="const", bufs=1))
        qkv = ctx.enter_context(tc.tile_pool(name="qkv", bufs=1))
        # wk_sb / wq_sb / aT0..aT3 ring-share two 16KB slots
        wbig = ctx.enter_context(tc.tile_pool(name="wbig", bufs=2))
        mm_ps = ctx.enter_context(tc.tile_pool(name="mm_ps", bufs=4, space="PSUM"))
        attv_ps = ctx.enter_context(tc.tile_pool(name="attv_ps", bufs=2, space="PSUM"))
        main = ctx.enter_context(tc.tile_pool(name="main", bufs=1))

        # ---- wk first: the first projection group needs it ----
        wk_sb = wbig.tile([P, N_EB, N_DB // 2, 2, P], FP8, tag="big16",
                          name="wk_sb")
        for eb in range(N_EB):
            nc.scalar.dma_start(out=wk_sb[:, eb], in_=wk[:, eb])
        wq_sb = wbig.tile([P, N_EB, N_DB // 2, 2, P], FP8, tag="big16",
                          name="wq_sb")
        wv_sb = qkv.tile([P, N_EC, N_DB, 512], BF16)

        # ---- constants ----
        eps_t = const.tile([P, 1], F32)
        nc.vector.memset(eps_t, EPS)
        ones_t = const.tile([P, 1], BF16)
        nc.vector.memset(ones_t, 1.0)
        ones_row = const.tile([1, P], F32)
        nc.vector.memset(ones_row, 1.0)
        bq_sb = const.tile([P, N_EB], F32)
        nc.sync.dma_start(out=bq_sb, in_=bq[:, :])
        bv_bc = const.tile([P, D], F32)
        nc.sync.dma_start(out=bv_bc, in_=bv[:, :])
        ident_sb = const.tile([P, P], F32)
        nc.sync.dma_start(out=ident_sb, in_=ident[:, :])
        # per-token rstd, [p, it] (token = it*128+p)
        rstd_t_all = const.tile([P, N_IT], F32)
        rstd_k_all = const.tile([P, N_IT], F32)
        rstd_v_all = const.tile([P, N_IT], F32)
        exp_sc_k = const.tile([P, N_IT], F32)     # EXP_SCALE * rstd_k
        rstd_bc = const.tile([P, T], F32)         # rstd_t broadcast over partitions

        # ---- persistent projection outputs ----
        qTp = qkv.tile([P, N_EB // 2, 2, T], qk_dt)
        kT = qkv.tile([P, N_EB, T], qk_dt)
        v_sb = qkv.tile([P, N_IT, D], BF16)

        RSTD = {"t": rstd_t_all, "k": rstd_k_all, "v": rstd_v_all}
        X = {"t": x_t, "k": x_k, "v": x_v}
        XT = {"t": xT_t, "k": xT_k, "v": xT_v}
        xtc = {}   # (stream, ic) -> xT chunk tile

        def emit_chunk_loads(sname, ic):
            if sname == "v":
                xc = main.tile([P, N_DB, 512], BF16, tag="xtc_v", bufs=2,
                               name=f"xtc_v_{ic}")
            else:
                xc = main.tile([P, N_DB // 2, 2, 512], FP8, tag=f"xtc_{sname}",
                               bufs=2, name=f"xtc_{sname}_{ic}")
            nc.gpsimd.dma_start(out=xc, in_=XT[sname][ic])
            xtc[(sname, ic)] = xc

        def emit_stats(sname, ic):
            """rstd for the 512 tokens of chunk ic (4 token tiles)."""
            x2 = main.tile([P, 4, D], BF16, tag="x_raw", bufs=3,
                           name=f"x_raw_{sname}_{ic}")
            nc.gpsimd.dma_start(
                out=x2,
                in_=X[sname][ts(ic, 512), :].rearrange("(four p) d -> p four d", p=P),
            )
            st = main.tile([P, 4, 2, 6], F32, tag="st", bufs=2,
                           name=f"st_{sname}_{ic}")
            for q in range(4):
                for sb in range(2):
                    nc.vector.bn_stats(out=st[:, q, sb, :], in_=x2[:, q, ts(sb, 512)])
            mv = main.tile([P, 4, 2], F32, tag="mv", bufs=2, name=f"mv_{sname}_{ic}")
            for q in range(4):
                nc.vector.bn_aggr(out=mv[:, q, :], in_=st[:, q])
            rstd = RSTD[sname]
            for q in range(4):
                nc.scalar.activation(
                    out=rstd[:, 4 * ic + q:4 * ic + q + 1], in_=mv[:, q, 1:2],
                    func=mybir.ActivationFunctionType.Abs_reciprocal_sqrt,
                    bias=eps_t,
                )
            if sname == "k":
                nc.scalar.mul(
                    out=exp_sc_k[:, ds(4 * ic, 4)], in_=rstd[:, ds(4 * ic, 4)],
                    mul=EXP_SCALE,
                )

        def emit_bc_row(ic):
            """rstd_t chunk row-extract via identity matmul + ACT copy to SBUF
            (gpsimd cannot read PSUM; ACT keeps it off the DVE FIFO)."""
            rowp = mm_ps.tile([1, 512], F32, tag="mm", name=f"rowp_{ic}")
            for il in range(4):
                nc.tensor.matmul(
                    rowp[:, ts(il, P)], lhsT=rstd_t_all[:, 4 * ic + il:4 * ic + il + 1],
                    rhs=ident_sb, start=True, stop=True,
                )
            row_sb = main.tile([1, 512], F32, tag="row_sb", bufs=2,
                               name=f"row_sb_{ic}")
            nc.scalar.copy(out=row_sb, in_=rowp)
            return row_sb

        def emit_bc_spread(ic, row_sb):
            """rstd_bc[:, chunk ic] = ones (x) row outer product."""
            bcp = mm_ps.tile([P, 512], F32, tag="mm", name=f"bcp_{ic}")
            nc.tensor.matmul(bcp, lhsT=ones_row, rhs=row_sb, start=True, stop=True)
            nc.scalar.copy(out=rstd_bc[:, ts(ic, 512)], in_=bcp)

        # fp8 proj psum = sum (16x)(2048w) = 32768*y; dequant to 32*y.
        PROJ_DEQ = QK_SCALE / (16.0 * 2048.0)

        def proj_k_chunk(ic):
            chunk = xtc[("k", ic)]
            with nc.named_scope(f"pj_k{ic}"):
                for eb in range(N_EB):
                    ps = mm_ps.tile([P, 512], F32, tag="mm", name=f"ps_k_{eb}_{ic}")
                    for dbp in range(N_DB // 2):
                        nc.tensor.matmul(
                            ps, lhsT=wk_sb[:, eb, dbp], rhs=chunk[:, dbp],
                            start=(dbp == 0), stop=(dbp == N_DB // 2 - 1),
                            perf_mode=mybir.MatmulPerfMode.DoubleRow,
                        )
                    # k carries no rstd (folded into the EXP scale) and no
                    # bias (cancels in softmax): plain x32 -> fp8.
                    nc.scalar.activation(
                        out=kT[:, eb, ts(ic, 512)], in_=ps,
                        func=mybir.ActivationFunctionType.Identity,
                        scale=PROJ_DEQ,
                    )

        def proj_q_chunk(ic):
            chunk = xtc[("t", ic)]
            with nc.named_scope(f"pj_q{ic}"):
                for eb in range(N_EB):
                    ps = mm_ps.tile([P, 512], F32, tag="mm", name=f"ps_q_{eb}_{ic}")
                    for dbp in range(N_DB // 2):
                        nc.tensor.matmul(
                            ps, lhsT=wq_sb[:, eb, dbp], rhs=chunk[:, dbp],
                            start=(dbp == 0), stop=(dbp == N_DB // 2 - 1),
                            perf_mode=mybir.MatmulPerfMode.DoubleRow,
                        )
                    # q = rstd_tok * y  (free-axis scale -> bc tile), then
                    # ACT adds bias (x32) and quantizes to fp8.
                    tmp = main.tile([P, 512], BF16, tag="qtmp", bufs=2,
                                    name=f"qtmp_{eb}_{ic}")
                    nc.vector.tensor_tensor(
                        out=tmp, in0=ps, in1=rstd_bc[:, ts(ic, 512)],
                        op=mybir.AluOpType.mult,
                    )
                    nc.scalar.activation(
                        out=qTp[:, eb // 2, eb % 2, ts(ic, 512)], in_=tmp,
                        func=mybir.ActivationFunctionType.Identity,
                        bias=bq_sb[:, eb:eb + 1], scale=PROJ_DEQ,
                    )

        def proj_v_chunk(jc):
            chunk = xtc[("v", jc)]
            with nc.named_scope(f"pj_v{jc}"):
                for jl in range(4):
                    jt = 4 * jc + jl
                    for ec in range(N_EC):
                        ps = mm_ps.tile([P, 512], F32, tag="mm",
                                        name=f"ps_v_{jt}_{ec}")
                        for db in range(N_DB):
                            nc.tensor.matmul(
                                ps, lhsT=chunk[:, db, ts(jl, P)],
                                rhs=wv_sb[:, ec, db, :],
                                start=(db == 0), stop=(db == N_DB - 1),
                            )
                        nc.vector.scalar_tensor_tensor(
                            out=v_sb[:, jt, ts(ec, 512)], in0=ps,
                            scalar=rstd_v_all[:, jt:jt + 1],
                            in1=bv_bc[:, ts(ec, 512)],
                            op0=mybir.AluOpType.mult, op1=mybir.AluOpType.add,
                        )

        # ---- emission: per chunk, loads -> stats -> (k-proj, bc, q-proj,
        # v-proj). PE is fed by linear DMA only; stats run concurrently and
        # are consumed at dequant time.
        # ~3us of throwaway matmuls, dependent only on memsets (never on a
        # DMA): spans the HAM activity window so the real projections start
        # at 2.4 GHz.
        warm_rhs = const.tile([P, P], BF16)
        nc.vector.memset(warm_rhs, 0.0)
        warm_ps = mm_ps.tile([1, P], F32, tag="mm", name="warm_ps")
        for w in range(30):
            nc.tensor.matmul(warm_ps, lhsT=ones_t, rhs=warm_rhs,
                             start=(w == 0), stop=(w == 29))

        def emit_loads(ic):
            with nc.named_scope(f"ld_{ic}"):
                for sname in ("k", "t", "v"):
                    emit_chunk_loads(sname, ic)
                    emit_stats(sname, ic)

        def emit_projs(ic):
            proj_k_chunk(ic)
            row = emit_bc_row(ic)
            emit_bc_spread(ic, row)
            proj_q_chunk(ic)
            if ic < N_IC - 1:
                proj_v_chunk(ic)

        # loads/stats run one chunk AHEAD of the projections: chunk c+1's
        # bn_stats sit in the DVE FIFO before chunk c's dequants (which stall
        # on PE psums), so rstd(c+1) is always ready when the PE needs it.
        emit_loads(0)
        # gpsimd queue = DMA need-order: behind chunk-0 inputs, ahead of
        # chunk-1 inputs. (On the scalar queue these issues sat behind
        # sem-stalled rstd ops -> 15us PE gap at q0.)
        for eb in range(N_EB):
            nc.gpsimd.dma_start(out=wq_sb[:, eb], in_=wq[:, eb])
        for ec in range(N_EC):
            nc.gpsimd.dma_start(out=wv_sb[:, ec], in_=wv[:, ec])
        emit_loads(1)
        emit_projs(0)
        emit_loads(2)
        emit_projs(1)
        emit_loads(3)
        emit_projs(2)
        emit_projs(3)

        # ---- attention ----
        def scores(ic):
            aT = wbig.tile([P, N_IT, 512], BF16, tag="big16", name=f"aT_{ic}")
            with nc.named_scope(f"scores_{ic}"):
                for jt in range(N_IT):
                    ps = mm_ps.tile([P, 512], F32, tag="mm", name=f"ps_s_{ic}_{jt}")
                    if USE_FP8_SCORES:
                        for ebp in range(N_EB // 2):
                            nc.tensor.matmul(
                                ps, lhsT=kT[:, ds(2 * ebp, 2), ts(jt, P)],
                                rhs=qTp[:, ebp, :, ts(ic, 512)],
                                start=(ebp == 0), stop=(ebp == N_EB // 2 - 1),
                                perf_mode=mybir.MatmulPerfMode.DoubleRow,
                            )
                    else:
                        for eb in range(N_EB):
                            nc.tensor.matmul(
                                ps, lhsT=kT[:, eb, ts(jt, P)],
                                rhs=qTp[:, eb // 2, eb % 2, ts(ic, 512)],
                                start=(eb == 0), stop=(eb == N_EB - 1),
                            )
                    # rstd of the k tokens rides the per-partition exp scale
                    nc.scalar.activation(
                        out=aT[:, jt, :], in_=ps,
                        func=mybir.ActivationFunctionType.Exp,
                        scale=exp_sc_k[:, jt:jt + 1],
                    )
            return aT

        def attv(ic, aT, zfirst=False):
            def z_block():
                zp = mm_ps.tile([P, 512], F32, tag="mm", name=f"zp_{ic}")
                for jt in range(N_IT):
                    nc.tensor.matmul(zp[0:1, :], lhsT=ones_t, rhs=aT[:, jt, :],
                                     start=(jt == 0), stop=(jt == N_IT - 1))
                z_sb = main.tile([1, 512], F32, tag="row_sb", bufs=2,
                                 name=f"z_sb_{ic}")
                nc.vector.tensor_copy(out=z_sb, in_=zp[0:1, :])
                nc.sync.dma_start(out=out_z[ic:ic + 1, :], in_=z_sb)

            with nc.named_scope(f"attv_{ic}"):
                if zfirst:
                    z_block()
                for isub in range(4):
                    ou = attv_ps.tile([P, D], F32, tag="ou", name=f"ou_{ic}_{isub}")
                    split = zfirst and isub == 3   # kernel tail: drain by halves
                    o_sb = main.tile([P, D], BF16, tag="o_sb", bufs=2,
                                     name=f"o_{ic}_{isub}")
                    for ec in range(N_EC):
                        for jt in range(N_IT):
                            nc.tensor.matmul(
                                ou[:, ts(ec, 512)], lhsT=aT[:, jt, ts(isub, P)],
                                rhs=v_sb[:, jt, ts(ec, 512)],
                                start=(jt == 0), stop=(jt == N_IT - 1))
                        if split:
                            nc.vector.tensor_copy(out=o_sb[:, ts(ec, 512)],
                                                  in_=ou[:, ts(ec, 512)])
                            nc.sync.dma_start(
                                out=out[ts(ic * 4 + isub, P), ts(ec, 512)],
                                in_=o_sb[:, ts(ec, 512)])
                    if not split:
                        nc.vector.tensor_copy(out=o_sb, in_=ou)
                        nc.sync.dma_start(out=out[ts(ic * 4 + isub, P), :], in_=o_sb)
                if not zfirst:
                    z_block()

        aT0 = scores(0)
        proj_v_chunk(N_IC - 1)
        aT1 = scores(1)
        attv(0, aT0)
        aT2 = scores(2)
        attv(1, aT1)
        aT3 = scores(3)
        attv(2, aT2)
        attv(3, aT3, zfirst=True)

    nc.compile()
    return nc


_NC_CACHE = None


def _get_module():
    global _NC_CACHE
    if _NC_CACHE is None:
        _NC_CACHE = build_module()
    return _NC_CACHE


def kernel(target, source_k, source_v, Wq, bq, Wk, bk, Wv, bv,
           g_t, b_t, g_k, b_k, g_v, b_v):
    target = np.asarray(target, dtype=np.float32)
    source_k = np.asarray(source_k, dtype=np.float32)
    source_v = np.asarray(source_v, dtype=np.float32)
    Wq = np.asarray(Wq, dtype=np.float32); bq = np.asarray(bq, dtype=np.float32)
    Wk = np.asarray(Wk, dtype=np.float32); bk = np.asarray(bk, dtype=np.float32)
    Wv = np.asarray(Wv, dtype=np.float32); bv = np.asarray(bv, dtype=np.float32)
    g_t = np.asarray(g_t, dtype=np.float32); b_t = np.asarray(b_t, dtype=np.float32)
    g_k = np.asarray(g_k, dtype=np.float32); b_k = np.asarray(b_k, dtype=np.float32)
    g_v = np.asarray(g_v, dtype=np.float32); b_v = np.asarray(b_v, dtype=np.float32)

    bf16 = ml_dtypes.bfloat16
    f8 = ml_dtypes.float8_e4m3

    # Fold layernorm affine into W/b, and mean-centering into W:
    #   LN_affine(x) @ W.T + b == (x @ C) diag(rstd-less...) -- precisely:
    #   (x - mu) @ Wg.T * rstd + b_eff, with Wg = W*g, b_eff = b + W @ b_ln,
    #   and (x - mu) @ Wg.T == x @ (Wg.T - colmean(Wg.T)).
    # q/k weights ship fp8 (x2048) with d-block pairs plane-interleaved for
    # DoubleRow; v stays bf16.
    def prep_qk(W, g):
        wT = (W * g[None, :]).T
        wT = wT - wT.mean(axis=0, keepdims=True)
        w8 = np.clip(wT * 2048.0, -448, 448).astype(f8)
        # [p, eb, dbp, h, el] with d = (2*dbp+h)*128+p, e = eb*128+el
        return np.ascontiguousarray(
            w8.reshape(N_DB // 2, 2, P, N_EB, P).transpose(2, 3, 0, 1, 4))

    def prep_v(W, g):
        wT = (W * g[None, :]).T
        wT = wT - wT.mean(axis=0, keepdims=True)
        wT = wT.astype(bf16)
        return np.ascontiguousarray(
            wT.reshape(N_DB, P, N_EC, 512).transpose(1, 2, 0, 3))

    wqT = prep_qk(Wq, g_t)
    wkT = prep_qk(Wk, g_k)
    wvT = prep_v(Wv, g_v)
    bq_f = np.ascontiguousarray(
        ((bq + Wq @ b_t) * QK_SCALE).reshape(8, 128).T)
    # b_k is dropped on device: softmax over j cancels the q_i . b_k shift.
    bv_f = np.ascontiguousarray(
        np.broadcast_to(bv + Wv @ b_v, (128, D)))
    ident = np.eye(P, dtype=np.float32)

    t_bf = target.astype(bf16)
    k_bf = source_k.astype(bf16)
    v_bf = source_v.astype(bf16)

    def prep_xT_f8(xb):
        # fp8 x16, [ic, p, dbp, h, i] with x row ic*512+i, col (2*dbp+h)*128+p
        x8 = np.clip(xb.astype(np.float32) * 16.0, -448, 448).astype(f8)
        return np.ascontiguousarray(
            x8.T.reshape(N_DB // 2, 2, P, N_IC, 512).transpose(3, 2, 0, 1, 4))

    def prep_xT(xb):
        # [ic, p, db, i] with x row ic*512+i, col db*128+p
        return np.ascontiguousarray(
            xb.T.reshape(N_DB, P, N_IC, 512).transpose(2, 1, 0, 3))

    nc = _get_module()
    in_maps = []
    for b in range(B):
        in_maps.append({
            "x_t": np.ascontiguousarray(t_bf[b]),
            "x_k": np.ascontiguousarray(k_bf[b]),
            "x_v": np.ascontiguousarray(v_bf[b]),
            "xT_t": prep_xT_f8(t_bf[b]),
            "xT_k": prep_xT_f8(k_bf[b]),
            "xT_v": prep_xT(v_bf[b]),
            "wq": wqT, "wk": wkT, "wv": wvT,
            "bq": bq_f, "bv": bv_f, "ident": ident,
        })

    res = run_bass_kernel_spmd(nc, in_maps, core_ids=list(range(B)),
                               trace=bool(int(os.environ.get("KERNEL_TRACE", "0"))))
    outs = []
    for b in range(B):
        ou = np.asarray(res.results[b]["out"], dtype=np.float32)
        z = np.asarray(res.results[b]["out_z"], dtype=np.float32).reshape(T)
        outs.append(ou / z[:, None])
    out = np.stack(outs, axis=0)
    kernel.last_results = res
    return out
